# revision 1
# baseline (speedup 1.0000x reference)
"""Trainium2 Bass kernel: AttentionBlock (B=2, S=2048, D=1024, H=16) on 8 NeuronCores.

Sharding: core c -> batch b=c//4, group rank g=c%4 (replica groups {0-3}, {4-7}).
Per core:
  - computes K,V for its 4 heads over all 2048 tokens of its batch
  - AllGathers K and V (bf16) within its 4-core group
  - computes Q for its 512-token slice (all 16 heads)
  - full attention + output projection + residual + LayerNorm for its slice
All on-chip tensors use a feature-on-partition ("transposed") layout; the host
pre-transposes/casts the inputs and transposes the per-core outputs back.
Softmax skips max-subtraction (scores ~ N(0,1)); the denominator comes from an
appended ones-column in V, so it costs no extra TensorE time. ScalarE (exp) is
the bottleneck engine in the attention phase, so the schedule minimizes its
idle time: K is projected and gathered first (AG-K gates the first scores), V
follows in two half-gathers that land before the PV matmuls need them, and the
Q projection is interleaved with attention head pairs. The S->exp->PV chain is
software-pipelined so the PE stream never stalls on ScalarE.
"""

import sys

for _p in (
    "/root/.axon_site",
    "/root/.axon_site/_ro/trn_rl_repo",
    "/root/.axon_site/_ro/pypackages",
    "/opt/trn_rl_repo",
    "/opt/pypackages",
):
    if _p not in sys.path:
        sys.path.append(_p)

import numpy as np
import ml_dtypes
from contextlib import ExitStack

import concourse.bass as bass
import concourse.mybir as mybir
import concourse.tile as tile
from concourse import bacc
from concourse.bass_utils import run_bass_kernel_spmd

F32 = mybir.dt.float32
BF16 = mybir.dt.bfloat16
BF16_NP = ml_dtypes.bfloat16
AF = mybir.ActivationFunctionType
ALU = mybir.AluOpType

P = 128
B = 2
S = 2048          # tokens per batch
D = 1024
H = 16
HD = 64
TOK = 512         # query tokens per core
DC = D // P       # 8 feature chunks
KC = S // P       # 16 key chunks
NPAIR = KC // 2   # key chunks processed in exp-merged pairs
NCORES = 8
GSIZE = 4
DH_OWN = 256      # head dims owned per core (4 heads)
EPS = 1e-5
RG = [[0, 1, 2, 3], [4, 5, 6, 7]]
SCALE = 1.0 / np.sqrt(HD)


def _body(nc, tc, io, profile=False):
    (xt, xqb, xq32, wq, wk, wv, wp, bq, bk, bv, bp, lng, lnb, out_ext) = io

    with ExitStack() as ctx:
        consts = ctx.enter_context(tc.tile_pool(name="consts", bufs=1))
        wbig = ctx.enter_context(tc.tile_pool(name="wbig", bufs=1))
        bigx = ctx.enter_context(tc.tile_pool(name="bigx", bufs=1))
        sc_ps = ctx.enter_context(tc.tile_pool(name="sc_ps", bufs=4, space="PSUM"))
        mm_ps = ctx.enter_context(tc.tile_pool(name="mm_ps", bufs=2, space="PSUM"))
        o_ps = ctx.enter_context(tc.tile_pool(name="o_ps", bufs=2, space="PSUM"))
        stage = ctx.enter_context(tc.tile_pool(name="stage", bufs=3))
        ptp = ctx.enter_context(tc.tile_pool(name="ptp", bufs=8))
        small = ctx.enter_context(tc.tile_pool(name="small", bufs=1))
        repp = ctx.enter_context(tc.tile_pool(name="repp", bufs=2))
        tmp = ctx.enter_context(tc.tile_pool(name="tmp", bufs=3))
        dram = ctx.enter_context(tc.tile_pool(name="dram", bufs=1, space="DRAM"))

        wk_sb = consts.tile([P, DC, DH_OWN], BF16)
        nc.sync.dma_start(wk_sb[:], wk.rearrange("(c p) n -> p c n", p=P))
        wv_sb = consts.tile([P, DC, DH_OWN], BF16)
        nc.sync.dma_start(wv_sb[:], wv.rearrange("(c p) n -> p c n", p=P))
        bk_sb = consts.tile([P, 2], F32)
        nc.sync.dma_start(bk_sb[:], bk.rearrange("(c p) -> p c", p=P))

        # token-half-major load: the first K-proj accumulation groups need the
        # leading columns of ALL feature chunks, so land those first
        xt_sb = bigx.tile([P, DC, S], BF16, tag="bigx", name="xt_sb")
        xt_r = xt.rearrange("(c p) t -> p c t", p=P)
        for q in range(2):
            for c in range(DC):
                nc.sync.dma_start(
                    xt_sb[:, c, q * 1024:(q + 1) * 1024],
                    xt_r[:, c, q * 1024:(q + 1) * 1024],
                )

        # K projection FIRST: AG-K gates the first scores/exp, so it must hit
        # the collective queue as early as possible (~12us earlier than if V
        # went first).
        kin = dram.tile([DH_OWN, S], BF16)
        kout = dram.tile([D, S], BF16)
        for oc in range(2):
            for t4 in range(4):
                ps = mm_ps.tile([P, TOK], F32, tag="mm", name="ps_k")
                for c in range(DC):
                    nc.tensor.matmul(
                        ps[:],
                        lhsT=wk_sb[:, c, oc * P:(oc + 1) * P],
                        rhs=xt_sb[:, c, t4 * 512:(t4 + 1) * 512],
                        start=(c == 0),
                        stop=(c == DC - 1),
                    )
                kst = stage.tile([P, 512], BF16, tag="kst")
                nc.vector.tensor_scalar_add(kst[:], ps[:], bk_sb[:, oc:oc + 1])
                nc.sync.dma_start(kin[oc * P:(oc + 1) * P, t4 * 512:(t4 + 1) * 512], kst[:])
        if not profile:
            nc.gpsimd.collective_compute(
                "AllGather", ALU.bypass, replica_groups=RG,
                ins=[kin.opt()], outs=[kout.opt()],
            )
        kt_sb = consts.tile([P, DC, S], BF16)
        for c in range(DC):
            if profile:
                nc.sync.dma_start(kt_sb[:, c, :], kin[(c % 2) * P:(c % 2 + 1) * P, :])
            else:
                nc.sync.dma_start(kt_sb[:, c, :], kout[c * P:(c + 1) * P, :])

        # V projection in two token halves, each with its own AllGather, so
        # the first half's gather lands before the first PV matmuls need it.
        vin = dram.tile([S, DH_OWN], BF16)
        vouts = [dram.tile([GSIZE * (S // 2), DH_OWN], BF16, name=f"vout{i}")
                 for i in range(2)]
        v_sb = consts.tile([P, KC, H, HD + 1], BF16)
        nc.vector.memset(v_sb[:, :, :, HD:HD + 1], 1.0)
        HS = S // 2
        for vh in range(2):
            for t8 in range(KC // 2):
                t16 = vh * (KC // 2) + t8
                ps = mm_ps.tile([P, TOK], F32, tag="mm", name="ps_v")
                for c in range(DC):
                    nc.tensor.matmul(
                        ps[:, :DH_OWN],
                        lhsT=xt_sb[:, c, t16 * P:(t16 + 1) * P],
                        rhs=wv_sb[:, c, :],
                        start=(c == 0),
                        stop=(c == DC - 1),
                    )
                vst = stage.tile([P, DH_OWN], BF16, tag="vst")
                nc.vector.tensor_copy(vst[:], ps[:, :DH_OWN])
                nc.sync.dma_start(vin[t16 * P:(t16 + 1) * P, :], vst[:])
            if not profile:
                nc.gpsimd.collective_compute(
                    "AllGather", ALU.bypass, replica_groups=RG,
                    ins=[vin[vh * HS:(vh + 1) * HS, :].opt()],
                    outs=[vouts[vh].opt()],
                )
        # readbacks after both AG triggers so the second trigger is not stuck
        # behind gather-waiting DMAs in the Pool instruction stream
        for vh in range(2):
            for r in range(GSIZE):
                for lh in range(4):
                    vsrc = (vin[vh * HS:(vh + 1) * HS, :] if profile
                            else vouts[vh][r * HS:(r + 1) * HS, :])
                    src = vsrc[:, lh * HD:(lh + 1) * HD].rearrange(
                        "(kc p) d -> p kc d", p=P
                    )
                    # SWDGE queue: keeps the kt readback (HWDGE) out of the
                    # head-of-line shadow of these transfers
                    nc.gpsimd.dma_start(
                        v_sb[:, vh * (KC // 2):(vh + 1) * (KC // 2),
                             r * 4 + lh, 0:HD],
                        src,
                    )

        # Q-path inputs (needed from ~35us on; kept off the critical K DMAs)
        xqb_sb = consts.tile([P, DC, TOK], BF16)
        nc.sync.dma_start(xqb_sb[:], xqb.rearrange("(c p) t -> p c t", p=P))
        wq_sb = wbig.tile([P, DC, D], BF16, tag="w")
        nc.sync.dma_start(wq_sb[:], wq.rearrange("(c p) n -> p c n", p=P))
        bq_sb = consts.tile([P, DC], F32)
        nc.sync.dma_start(bq_sb[:], bq.rearrange("(c p) -> p c", p=P))
        bv_sb = consts.tile([P, DC], F32)
        nc.sync.dma_start(bv_sb[:], bv.rearrange("(c p) -> p c", p=P))

        # -------- Q projection interleaved with attention: after Q chunk oc
        # -------- is done, heads 2*oc and 2*oc+1 can run, so the exp pipeline
        # -------- on ScalarE starts as soon as AG-K lands instead of after
        # -------- all of Q. The PE stream is software-pipelined: S(kc+1) is
        # -------- emitted before PV(kc) so PE never waits on the exp.
        qt_sb = consts.tile([P, DC, TOK], BF16)
        ot_sb = consts.tile([P, DC, TOK], BF16)

        def q_proj_chunk(oc):
            ps = mm_ps.tile([P, TOK], F32, tag="mm", name="ps_q")
            for c in range(DC):
                nc.tensor.matmul(
                    ps[:],
                    lhsT=wq_sb[:, c, oc * P:(oc + 1) * P],
                    rhs=xqb_sb[:, c, :],
                    start=(c == 0),
                    stop=(c == DC - 1),
                )
            nc.vector.tensor_scalar_add(qt_sb[:, oc, :], ps[:], bq_sb[:, oc:oc + 1])

        def attention_head(h):
            jq, off = h // 2, (h % 2) * HD
            qh = qt_sb[off:off + HD, jq, :]
            po = o_ps.tile([HD + 1, TOK], F32, tag="o", name="po")
            prev_pt, prev_kc = None, -1
            for kc in range(KC):
                ps_s = sc_ps.tile([P, TOK], F32, tag="sc", name="ps_s")
                nc.tensor.matmul(
                    ps_s[:],
                    lhsT=kt_sb[off:off + HD, jq, kc * P:(kc + 1) * P],
                    rhs=qh,
                    start=True,
                    stop=True,
                )
                if prev_pt is not None:
                    nc.tensor.matmul(
                        po[:], lhsT=v_sb[:, prev_kc, h, :], rhs=prev_pt[:],
                        start=(prev_kc == 0), stop=False, skip_group_check=True,
                    )
                pt = ptp.tile([P, TOK], BF16, tag="pt", name="pt")
                nc.scalar.activation(pt[:], ps_s[:], AF.Exp, scale=float(SCALE))
                prev_pt, prev_kc = pt, kc
            nc.tensor.matmul(
                po[:], lhsT=v_sb[:, KC - 1, h, :], rhs=prev_pt[:],
                start=False, stop=True, skip_group_check=True,
            )
            rden = small.tile([1, TOK], F32, tag="rden", name="rden", bufs=2)
            nc.vector.reciprocal(rden[:], po[HD:HD + 1, :])
            rep = repp.tile([HD, TOK], F32, tag="rep", name="rep")
            nc.gpsimd.partition_broadcast(rep[:], rden[:])
            nc.vector.tensor_tensor(ot_sb[off:off + HD, jq, :], po[0:HD, :], rep[:], ALU.mult)
            nc.vector.tensor_scalar_add(
                ot_sb[off:off + HD, jq, :], ot_sb[off:off + HD, jq, :],
                bv_sb[off:off + HD, jq:jq + 1],
            )

        for oc in range(DC):
            q_proj_chunk(oc)
            attention_head(2 * oc)
            attention_head(2 * oc + 1)

        # ---------------- output projection + residual + LayerNorm ----------------
        wp_sb = wbig.tile([P, DC, D], BF16, tag="w")
        nc.sync.dma_start(wp_sb[:], wp.rearrange("(c p) n -> p c n", p=P))
        # xt is dead once QKV projections finish; reuse its slot for the f32 slice
        xq32_sb = bigx.tile([P, DC, TOK], F32, tag="bigx", name="xq32_sb")
        nc.sync.dma_start(xq32_sb[:], xq32.rearrange("(c p) t -> p c t", p=P))
        bp_sb = consts.tile([P, DC], F32)
        nc.sync.dma_start(bp_sb[:], bp.rearrange("(c p) -> p c", p=P))
        lng_sb = consts.tile([P, DC], F32)
        nc.sync.dma_start(lng_sb[:], lng.rearrange("(c p) -> p c", p=P))
        lnb_sb = consts.tile([P, DC], F32)
        nc.sync.dma_start(lnb_sb[:], lnb.rearrange("(c p) -> p c", p=P))
        eps_sb = consts.tile([1, 1], F32)
        nc.vector.memset(eps_sb[:], EPS)
        ones_sb = consts.tile([P, 1], BF16)
        nc.vector.memset(ones_sb[:], 1.0)

        y_sb = xq32_sb  # reuse: y = xq + proj + bp, in place over the f32 x slice
        # Token-split epilogue: run proj + residual + LN per 256-token half so
        # the serial LN tail is half-length and overlaps the other half's
        # matmuls. LN stats accumulators live in the (now idle) attention-
        # output psum slots.
        HT = TOK // 2
        for th in range(2):
            tsl = slice(th * HT, (th + 1) * HT)
            ps_sum = o_ps.tile([1, HT], F32, tag="o", name="ps_sum")
            ps_sq = o_ps.tile([1, HT], F32, tag="o", name="ps_sq")
            for oc in range(DC):
                ps = sc_ps.tile([P, TOK], F32, tag="sc", name="ps_p")
                for c in range(DC):
                    nc.tensor.matmul(
                        ps[:, :HT],
                        lhsT=wp_sb[:, c, oc * P:(oc + 1) * P],
                        rhs=ot_sb[:, c, tsl],
                        start=(c == 0),
                        stop=(c == DC - 1),
                    )
                nc.vector.tensor_add(y_sb[:, oc, tsl], ps[:, :HT], y_sb[:, oc, tsl])
                nc.vector.tensor_scalar_add(y_sb[:, oc, tsl], y_sb[:, oc, tsl],
                                            bp_sb[:, oc:oc + 1])
                ybf = tmp.tile([P, HT], BF16, tag="ybf", name="ybf")
                nc.gpsimd.tensor_copy(ybf[:], y_sb[:, oc, tsl])
                ysq = tmp.tile([P, HT], BF16, tag="ysq", name="ysq")
                nc.vector.tensor_mul(ysq[:], y_sb[:, oc, tsl], y_sb[:, oc, tsl])
                nc.tensor.matmul(ps_sum[:], lhsT=ones_sb[:], rhs=ybf[:],
                                 start=(oc == 0), stop=(oc == DC - 1))
                nc.tensor.matmul(ps_sq[:], lhsT=ones_sb[:], rhs=ysq[:],
                                 start=(oc == 0), stop=(oc == DC - 1))

            mean = small.tile([1, HT], F32, tag="mean", name="mean", bufs=2)
            nc.vector.tensor_scalar_mul(mean[:], ps_sum[:], 1.0 / D)
            var = small.tile([1, HT], F32, tag="var", name="var", bufs=2)
            nc.vector.tensor_mul(var[:], mean[:], mean[:])
            ex2 = small.tile([1, HT], F32, tag="ex2", name="ex2", bufs=2)
            nc.vector.tensor_scalar_mul(ex2[:], ps_sq[:], 1.0 / D)
            nc.vector.tensor_sub(var[:], ex2[:], var[:])
            nc.scalar.activation(var[:], var[:], AF.Sqrt, bias=eps_sb[:])
            nc.vector.reciprocal(var[:], var[:])          # invstd
            nc.vector.tensor_mul(mean[:], mean[:], var[:])  # mean*invstd
            rep_is = repp.tile([P, HT], F32, tag="repbig", name="rep_is")
            nc.gpsimd.partition_broadcast(rep_is[:], var[:])
            rep_mi = repp.tile([P, HT], F32, tag="repbig", name="rep_mi")
            nc.gpsimd.partition_broadcast(rep_mi[:], mean[:])

            for oc in range(DC):
                t1 = tmp.tile([P, HT], F32, tag="t1", name="t1")
                nc.vector.tensor_mul(t1[:], y_sb[:, oc, tsl], rep_is[:])
                nc.vector.tensor_sub(t1[:], t1[:], rep_mi[:])
                nc.vector.tensor_scalar(
                    y_sb[:, oc, tsl], t1[:],
                    lng_sb[:, oc:oc + 1], lnb_sb[:, oc:oc + 1],
                    ALU.mult, ALU.add,
                )
                nc.sync.dma_start(out_ext[oc * P:(oc + 1) * P, tsl], y_sb[:, oc, tsl])


def build(profile=False):
    try:
        from concourse.bass_utils import axon_active
        debug = not axon_active()  # native NRT path wants debug buffers
    except Exception:
        debug = False
    nc = bacc.Bacc(
        "TRN2", target_bir_lowering=False, debug=debug,
        num_devices=1 if profile else NCORES,
    )
    xt = nc.dram_tensor("xt", [D, S], BF16, kind="ExternalInput")
    xqb = nc.dram_tensor("xqb", [D, TOK], BF16, kind="ExternalInput")
    xq32 = nc.dram_tensor("xq32", [D, TOK], F32, kind="ExternalInput")
    wq = nc.dram_tensor("wq", [D, D], BF16, kind="ExternalInput")
    wk = nc.dram_tensor("wk", [D, DH_OWN], BF16, kind="ExternalInput")
    wv = nc.dram_tensor("wv", [D, DH_OWN], BF16, kind="ExternalInput")
    wp = nc.dram_tensor("wp", [D, D], BF16, kind="ExternalInput")
    bq = nc.dram_tensor("bq", [D], F32, kind="ExternalInput")
    bk = nc.dram_tensor("bk", [DH_OWN], F32, kind="ExternalInput")
    bv = nc.dram_tensor("bv", [D], F32, kind="ExternalInput")
    bp = nc.dram_tensor("bp", [D], F32, kind="ExternalInput")
    lng = nc.dram_tensor("lng", [D], F32, kind="ExternalInput")
    lnb = nc.dram_tensor("lnb", [D], F32, kind="ExternalInput")
    out_ext = nc.dram_tensor("out", [D, TOK], F32, kind="ExternalOutput")

    io = (xt[:], xqb[:], xq32[:], wq[:], wk[:], wv[:], wp[:], bq[:], bk[:],
          bv[:], bp[:], lng[:], lnb[:], out_ext[:])
    with tile.TileContext(nc) as tc:
        _body(nc, tc, io, profile=profile)
    nc.compile()
    return nc


_NC = None


def _get_nc():
    global _NC
    if _NC is None:
        _NC = build()
    return _NC


def shard_inputs(inputs):
    x = np.asarray(inputs["x"], np.float32)
    Wq = np.asarray(inputs["Wq"], np.float32)
    Wk = np.asarray(inputs["Wk"], np.float32)
    Wv = np.asarray(inputs["Wv"], np.float32)
    Wp = np.asarray(inputs["Wp"], np.float32)
    bq = np.asarray(inputs["bq"], np.float32)
    bk = np.asarray(inputs["bk"], np.float32)
    bv = np.asarray(inputs["bv"], np.float32)
    bp = np.asarray(inputs["bp"], np.float32)
    lng = np.asarray(inputs["ln_g"], np.float32)
    lnb = np.asarray(inputs["ln_b"], np.float32)

    wq_b = np.ascontiguousarray(Wq).astype(BF16_NP)
    wp_b = np.ascontiguousarray(Wp).astype(BF16_NP)
    xt_b = [np.ascontiguousarray(x[b].T).astype(BF16_NP) for b in range(B)]

    in_maps = []
    for c in range(NCORES):
        b, g = c // GSIZE, c % GSIZE
        xq32 = np.ascontiguousarray(x[b, g * TOK:(g + 1) * TOK, :].T)
        in_maps.append({
            "xt": xt_b[b],
            "xqb": xq32.astype(BF16_NP),
            "xq32": xq32,
            "wq": wq_b,
            "wk": np.ascontiguousarray(Wk[:, g * DH_OWN:(g + 1) * DH_OWN]).astype(BF16_NP),
            "wv": np.ascontiguousarray(Wv[:, g * DH_OWN:(g + 1) * DH_OWN]).astype(BF16_NP),
            "wp": wp_b,
            "bq": bq,
            "bk": np.ascontiguousarray(bk[g * DH_OWN:(g + 1) * DH_OWN]),
            "bv": bv,
            "bp": bp,
            "lng": lng,
            "lnb": lnb,
        })
    return in_maps


def assemble(results):
    out = np.empty((B, S, D), np.float32)
    for c in range(NCORES):
        b, g = c // GSIZE, c % GSIZE
        out[b, g * TOK:(g + 1) * TOK, :] = results[c]["out"].T
    return out


def run(inputs, trace=False):
    nc = _get_nc()
    in_maps = shard_inputs(inputs)
    res = run_bass_kernel_spmd(nc, in_maps, core_ids=list(range(NCORES)), trace=trace)
    return assemble(res.results), res.exec_time_ns


def kernel(**inputs):
    out, _ = run(inputs)
    return out

